# revision 29
# baseline (speedup 1.0000x reference)
"""Trainium2 Bass kernel for nn_AsynBaseStem (sparse 7x7 conv + BN + ReLU +
scatter + 3x3/2 maxpool), 8-core data-parallel over output row bands.

v2: each core's 81-row band is split into two 41-row half-bands processed in
parallel on PE column groups (tile_position col 0/64), so every PSUM tile is
[128, 512] = two 512-pixel streams and all eviction/pool engine work runs at
128 partitions (half the per-pixel instruction cost of v1). Matmuls are
phase-batched (4x A-stationary, then 4x tail-stationary per 4-PSUM-bank
batch) so independent banks stream back-to-back on the PE.

Per core:
  - Host builds a [128, 2*(41*646+8)] bf16 operand table: per half-band,
    rows 0..125 = column-shifted planar stripes, row 126 = inactive flag,
    row 127 = ones (bias row). BN scale/bias and the -1e9 flag weight are
    folded into duplicated [128,128]/[21,128] stationaries on the host.
  - Dense conv at every pixel: K=128 main + K=21 tail accumulating matmuls.
  - Eviction per tile: ACT relu-copies even columns, DVE maxes them with the
    PSUM odd columns (one PSUM operand max per op), both into full-length
    bf16 rings - row length 646 is even, so pair parity never straddles
    rows. The early ReLU propagates the final max-with-0 through the pool.
  - DVE does the per-row 3-col max and the 3-row max; gpsimd DMAs pooled
    row-pairs out as bf16 (host casts to f32).

kernel(**inputs) takes FULL unsharded inputs, returns [319, 319, 64] f32.
"""
import numpy as np
import ml_dtypes
from contextlib import ExitStack

H = W = 640
CIN, COUT = 3, 64
K, PAD = 7, 3
NCORES = 8
HROWS = 41            # dense rows per half-band
WPAD = W + 2 * PAD    # 646
NBH = HROWS * WPAD    # 26486 columns per half-band
NBHP = NBH + 8        # +pad so the tail matmul window (x+6) stays in bounds
HPOOL = 20            # pooled rows per half-band
QCOLS = 319
BN_EPS = 1e-5
NEG = -1.0e9
HCOLS = NBH // 2      # 13243 half-columns (pair stream) per half-band


def _build_bass():
    import concourse.bass as bass
    import concourse.mybir as mybir
    import concourse.tile as tile
    from concourse import bacc

    fp32 = mybir.dt.float32
    bf16 = mybir.dt.bfloat16

    nc = bacc.Bacc()
    t6_ext = nc.declare_dram_parameter("t6", [128, 2 * NBHP], bf16, isOutput=False)
    # host-folded stationaries (BN scale/bias + flag row baked in, duplicated
    # across both PE column groups)
    sa_ext = nc.declare_dram_parameter("statA", [128, 2 * COUT], bf16, isOutput=False)
    sb_ext = nc.declare_dram_parameter("statB", [21, 2 * COUT], bf16, isOutput=False)
    out_ext = nc.declare_dram_parameter("out", [128, HPOOL * 320], bf16, isOutput=True)

    with ExitStack() as ctx:
        tc = ctx.enter_context(tile.TileContext(nc))
        cpool = ctx.enter_context(tc.tile_pool(name="const", bufs=1))
        rowp = ctx.enter_context(tc.tile_pool(name="rowbufs", bufs=4))
        ringp = ctx.enter_context(tc.tile_pool(name="ring", bufs=1))
        psp = ctx.enter_context(tc.tile_pool(name="ps", bufs=8, space="PSUM"))

        C = COUT
        statA = cpool.tile([128, 2 * COUT], bf16)
        statB = cpool.tile([21, 2 * COUT], bf16)
        nc.sync.dma_start(statA[:], sa_ext[:])
        nc.sync.dma_start(statB[:], sb_ext[:])

        # ---- big operand table: small early chunks alternating across both
        # HWDGE queues (scalar queue is idle at start) so the PE pipeline
        # fills without waiting on one serial chunk stream
        t6 = cpool.tile([128, 2 * NBHP], bf16)
        bounds = [0, 1, 2, 4, 8, 14, 22, 32, HROWS]
        for ck in range(len(bounds) - 1):
            a = bounds[ck] * WPAD
            b = bounds[ck + 1] * WPAD if ck + 2 < len(bounds) else NBHP
            eng = nc.scalar if ck == 0 else nc.sync
            eng.dma_start(t6[:, a:b], t6_ext[:, a:b])
            eng.dma_start(t6[:, NBHP + a:NBHP + b], t6_ext[:, NBHP + a:NBHP + b])

        # ---- full-length bf16 pair-stream rings + pooled accumulator ----
        ering = ringp.tile([128, HCOLS + 1], bf16)   # even columns
        tring = ringp.tile([128, HCOLS + 1], bf16)   # max(even, odd)
        mring = ringp.tile([128, 8 * 320], bf16)     # per-row 3-col max
        pooled = ringp.tile([128, HPOOL * 320], bf16)

        NT = (NBH + 511) // 512  # 52 tile-pairs
        next_row = [0]

        def finish_rows(xb):
            while (next_row[0] + 1) * WPAD <= xb:
                r = next_row[0]
                next_row[0] += 1
                h = r * (WPAD // 2)
                mrow = mring[:, (r % 8) * 320:(r % 8) * 320 + 320]
                # m[c'] = max(t[c'], e[c'+1]) : 3-col window max for row r
                nc.vector.tensor_tensor(
                    out=mrow[:, 0:QCOLS], in0=tring[:, h:h + QCOLS],
                    in1=ering[:, h + 1:h + 1 + QCOLS], op=mybir.AluOpType.max)
                if r >= 2 and r % 2 == 0:
                    p = (r - 2) // 2
                    m0 = mring[:, ((r - 2) % 8) * 320:((r - 2) % 8) * 320 + 320]
                    m1 = mring[:, ((r - 1) % 8) * 320:((r - 1) % 8) * 320 + 320]
                    s01 = rowp.tile([128, 320], bf16, tag="s01")
                    nc.vector.tensor_tensor(out=s01[:], in0=m0[:], in1=m1[:],
                                            op=mybir.AluOpType.max)
                    po = pooled[:, p * 320:(p + 1) * 320]
                    nc.vector.tensor_tensor(out=po[:], in0=s01[:], in1=mrow[:],
                                            op=mybir.AluOpType.max)
                    # stream pooled row-pairs out (bf16; host casts to f32)
                    if p % 2 == 1:
                        pc = p // 2
                        nc.sync.dma_start(
                            out_ext[:, pc * 640:(pc + 1) * 640],
                            pooled[:, pc * 640:(pc + 1) * 640])

        for b in range(0, NT, 4):
            ks = range(b, min(b + 4, NT))
            pss = {}
            for k in ks:
                xa = 512 * k
                xb = min(xa + 512, NBH)
                ps = psp.tile([128, 512], fp32, tag="convps", name=f"ps{k}")
                pss[k] = (ps, xa, xb, xb - xa)
            # phase A: main matmuls (independent PSUM banks back-to-back)
            for k in ks:
                ps, xa, xb, wdt = pss[k]
                nc.tensor.matmul(ps[0:64, 0:wdt], statA[:, 0:C],
                                 t6[0:128, xa:xb], start=True, stop=False)
                nc.tensor.matmul(ps[64:128, 0:wdt], statA[:, C:2 * C],
                                 t6[0:128, NBHP + xa:NBHP + xb],
                                 start=True, stop=False)
            # phase B: tail matmuls
            for k in ks:
                ps, xa, xb, wdt = pss[k]
                nc.tensor.matmul(ps[0:64, 0:wdt], statB[0:21, 0:C],
                                 t6[0:21, xa + 6:xb + 6], start=False, stop=True)
                nc.tensor.matmul(ps[64:128, 0:wdt], statB[0:21, C:2 * C],
                                 t6[0:21, NBHP + xa + 6:NBHP + xb + 6],
                                 start=False, stop=True)
            # eviction: ACT relu-copies even cols into the ring (the relu
            # propagates the final max-with-0 through the pool chain), DVE
            # maxes them with the PSUM odd cols
            for k in ks:
                ps, xa, xb, wdt = pss[k]
                hw2 = wdt // 2
                ho = xa // 2
                nc.scalar.activation(ering[:, ho:ho + hw2], ps[:, 0:wdt:2],
                                     mybir.ActivationFunctionType.Relu)
                nc.vector.tensor_tensor(
                    out=tring[:, ho:ho + hw2], in0=ering[:, ho:ho + hw2],
                    in1=ps[:, 1:wdt:2], op=mybir.AluOpType.max)
                finish_rows(xb)

    nc.finalize()
    return nc


_NC_CACHE = None


def _get_nc():
    global _NC_CACHE
    if _NC_CACHE is None:
        _NC_CACHE = _build_bass()
    return _NC_CACHE


def build_in_maps(update_location, feature_map, weight, gamma, beta,
                  running_mean, running_var):
    fm = np.asarray(feature_map, np.float32)
    loc = np.asarray(update_location).astype(np.int64)
    wt = np.asarray(weight, np.float32)

    fm_pad = np.pad(fm, ((PAD, PAD), (PAD, PAD), (0, 0)))          # [646,646,3]
    # stripes B_T[t=(i,ch), r, c] = fm_pad[r+i, c, ch], r in 0..640 (row 640 pad)
    bt = np.zeros((21, H + 1, WPAD), np.float32)
    for i in range(K):
        for ch in range(CIN):
            bt[i * CIN + ch, 0:H, :] = fm_pad[i:i + H, :, ch]
    bt = bt.astype(ml_dtypes.bfloat16)

    # inactive flag = 1 where no site; indexed by output pixel (r, c) at
    # position c in the 646-pitch row; columns 640..645 stay inactive.
    flag = np.ones((H + 1, WPAD), np.float32)
    flag[loc[:, 0], loc[:, 1]] = 0.0
    flag[:, H:] = 1.0
    flag = flag.astype(ml_dtypes.bfloat16)

    # reordered weights W_re[(j,i,ch), o] = weight[i, j, ch, o]
    w_re = np.ascontiguousarray(
        wt.transpose(1, 0, 2, 3).reshape(147, COUT)).astype(np.float32)

    ones_half = np.ones((HROWS, WPAD), ml_dtypes.bfloat16)

    def build_half(r0):
        t6 = np.zeros((128, HROWS, WPAD), ml_dtypes.bfloat16)
        for j in range(6):
            sl = bt[:, r0:r0 + HROWS, :]
            t6[j * 21:(j + 1) * 21, :, :-j or None] = sl[:, :, j:]
        t6[126] = flag[r0:r0 + HROWS]
        t6[127] = ones_half
        t6p = np.zeros((128, NBHP), ml_dtypes.bfloat16)
        t6p[:, :NBH] = t6.reshape(128, NBH)
        return t6p

    # host-folded BN: inv = gamma*rsqrt(var+eps), bias = beta - mean*inv
    inv = (np.asarray(gamma, np.float32) /
           np.sqrt(np.asarray(running_var, np.float32) + BN_EPS))
    bias = np.asarray(beta, np.float32) - np.asarray(running_mean, np.float32) * inv
    sa = np.zeros((128, COUT), np.float32)
    sa[0:126] = w_re[0:126] * inv[None, :]
    sa[126] = NEG          # flag row: inactive pixels -> -1e9
    sa[127] = bias         # ones row: + BN bias
    statA = np.ascontiguousarray(
        np.concatenate([sa, sa], axis=1).astype(ml_dtypes.bfloat16))
    sb = w_re[126:147] * inv[None, :]
    statB = np.ascontiguousarray(
        np.concatenate([sb, sb], axis=1).astype(ml_dtypes.bfloat16))

    in_maps = []
    for k in range(NCORES):
        r0 = 80 * k
        t6p = np.concatenate([build_half(r0), build_half(r0 + 40)], axis=1)
        in_maps.append({"t6": np.ascontiguousarray(t6p),
                        "statA": statA, "statB": statB})
    return in_maps


def kernel(update_location, feature_map, weight, gamma, beta, running_mean,
           running_var):
    from concourse.bass_utils import run_bass_kernel_spmd

    in_maps = build_in_maps(update_location, feature_map, weight, gamma, beta,
                            running_mean, running_var)
    nc = _get_nc()
    res = run_bass_kernel_spmd(nc, in_maps, core_ids=list(range(NCORES)))
    # per-core out is [128, HPOOL*320] bf16: partition (h*64+ch), free (p*320+q)
    parts = []
    for k in range(NCORES):
        o = np.asarray(res.results[k]["out"], np.float32).reshape(
            2, COUT, HPOOL, 320)
        parts.append(o.transpose(0, 2, 3, 1).reshape(2 * HPOOL, 320, COUT)[:, :QCOLS, :])
    out = np.concatenate(parts, axis=0)[:QCOLS]
    return np.ascontiguousarray(out).astype(np.float32)


# revision 30
# speedup vs baseline: 1.2235x; 1.2235x over previous
"""Trainium2 Bass kernel for nn_AsynBaseStem (sparse 7x7 conv + BN + ReLU +
scatter + 3x3/2 maxpool), 8-core data-parallel over output row bands.

v2: each core's 81-row band is split into two 41-row half-bands processed in
parallel on PE column groups (tile_position col 0/64), so every PSUM tile is
[128, 512] = two 512-pixel streams and all eviction/pool engine work runs at
128 partitions (half the per-pixel instruction cost of v1). Matmuls are
phase-batched (4x A-stationary, then 4x tail-stationary per 4-PSUM-bank
batch) so independent banks stream back-to-back on the PE.

Per core:
  - Host builds a [128, 2*(41*646+8)] bf16 operand table: per half-band,
    rows 0..125 = column-shifted planar stripes, row 126 = inactive flag,
    row 127 = ones (bias row). BN scale/bias and the -1e9 flag weight are
    folded into duplicated [128,128]/[21,128] stationaries on the host.
  - Dense conv at every pixel: K=128 main + K=21 tail accumulating matmuls.
  - Eviction per tile: ACT relu-copies even columns, DVE maxes them with the
    PSUM odd columns (one PSUM operand max per op), both into full-length
    bf16 rings - row length 646 is even, so pair parity never straddles
    rows. The early ReLU propagates the final max-with-0 through the pool.
  - DVE does the per-row 3-col max and the 3-row max; gpsimd DMAs pooled
    row-pairs out as bf16 (host casts to f32).

kernel(**inputs) takes FULL unsharded inputs, returns [319, 319, 64] f32.
"""
import numpy as np
import ml_dtypes
from contextlib import ExitStack

H = W = 640
CIN, COUT = 3, 64
K, PAD = 7, 3
NCORES = 8
HROWS = 41            # dense rows per half-band
WPAD = W + 2 * PAD    # 646
NBH = HROWS * WPAD    # 26486 columns per half-band
NBHP = NBH + 8        # +pad so the tail matmul window (x+6) stays in bounds
HPOOL = 20            # pooled rows per half-band
QCOLS = 319
BN_EPS = 1e-5
NEG = -1.0e9
HCOLS = NBH // 2      # 13243 half-columns (pair stream) per half-band


def _build_bass():
    import concourse.bass as bass
    import concourse.mybir as mybir
    import concourse.tile as tile
    from concourse import bacc

    fp32 = mybir.dt.float32
    bf16 = mybir.dt.bfloat16

    nc = bacc.Bacc()
    t6_ext = nc.declare_dram_parameter("t6", [128, 2 * NBHP], bf16, isOutput=False)
    # host-folded stationaries (BN scale/bias + flag row baked in, duplicated
    # across both PE column groups)
    sa_ext = nc.declare_dram_parameter("statA", [128, 2 * COUT], bf16, isOutput=False)
    sb_ext = nc.declare_dram_parameter("statB", [21, 2 * COUT], bf16, isOutput=False)
    out_ext = nc.declare_dram_parameter("out", [128, HPOOL * 320], bf16, isOutput=True)

    with ExitStack() as ctx:
        tc = ctx.enter_context(tile.TileContext(nc))
        cpool = ctx.enter_context(tc.tile_pool(name="const", bufs=1))
        rowp = ctx.enter_context(tc.tile_pool(name="rowbufs", bufs=4))
        ringp = ctx.enter_context(tc.tile_pool(name="ring", bufs=1))
        psp = ctx.enter_context(tc.tile_pool(name="ps", bufs=8, space="PSUM"))

        C = COUT
        statA = cpool.tile([128, 2 * COUT], bf16)
        statB = cpool.tile([21, 2 * COUT], bf16)
        nc.sync.dma_start(statA[:], sa_ext[:])
        nc.sync.dma_start(statB[:], sb_ext[:])

        # ---- big operand table: small early chunks alternating across both
        # HWDGE queues (scalar queue is idle at start) so the PE pipeline
        # fills without waiting on one serial chunk stream
        t6 = cpool.tile([128, 2 * NBHP], bf16)
        bounds = [0, 2, 4, 8, 14, 22, 32, HROWS]
        for ck in range(len(bounds) - 1):
            a = bounds[ck] * WPAD
            b = bounds[ck + 1] * WPAD if ck + 2 < len(bounds) else NBHP
            eng = nc.scalar if ck == 0 else nc.sync
            eng.dma_start(t6[:, a:b], t6_ext[:, a:b])
            eng.dma_start(t6[:, NBHP + a:NBHP + b], t6_ext[:, NBHP + a:NBHP + b])

        # ---- full-length bf16 pair-stream rings + pooled accumulator ----
        ering = ringp.tile([128, HCOLS + 1], bf16)   # even columns
        tring = ringp.tile([128, HCOLS + 1], bf16)   # max(even, odd)
        mring = ringp.tile([128, 8 * 320], bf16)     # per-row 3-col max
        pooled = ringp.tile([128, HPOOL * 320], bf16)

        NT = (NBH + 511) // 512  # 52 tile-pairs
        next_row = [0]

        def finish_rows(xb):
            while (next_row[0] + 1) * WPAD <= xb:
                r = next_row[0]
                next_row[0] += 1
                h = r * (WPAD // 2)
                mrow = mring[:, (r % 8) * 320:(r % 8) * 320 + 320]
                # m[c'] = max(t[c'], e[c'+1]) : 3-col window max for row r
                nc.vector.tensor_tensor(
                    out=mrow[:, 0:QCOLS], in0=tring[:, h:h + QCOLS],
                    in1=ering[:, h + 1:h + 1 + QCOLS], op=mybir.AluOpType.max)
                if r >= 2 and r % 2 == 0:
                    p = (r - 2) // 2
                    m0 = mring[:, ((r - 2) % 8) * 320:((r - 2) % 8) * 320 + 320]
                    m1 = mring[:, ((r - 1) % 8) * 320:((r - 1) % 8) * 320 + 320]
                    s01 = rowp.tile([128, 320], bf16, tag="s01")
                    nc.vector.tensor_tensor(out=s01[:], in0=m0[:], in1=m1[:],
                                            op=mybir.AluOpType.max)
                    po = pooled[:, p * 320:(p + 1) * 320]
                    nc.vector.tensor_tensor(out=po[:], in0=s01[:], in1=mrow[:],
                                            op=mybir.AluOpType.max)
                    # stream pooled row-pairs out (bf16; host casts to f32)
                    if p % 2 == 1:
                        pc = p // 2
                        nc.sync.dma_start(
                            out_ext[:, pc * 640:(pc + 1) * 640],
                            pooled[:, pc * 640:(pc + 1) * 640])

        for b in range(0, NT, 4):
            ks = range(b, min(b + 4, NT))
            pss = {}
            for k in ks:
                xa = 512 * k
                xb = min(xa + 512, NBH)
                ps = psp.tile([128, 512], fp32, tag="convps", name=f"ps{k}")
                pss[k] = (ps, xa, xb, xb - xa)
            # phase A: main matmuls (independent PSUM banks back-to-back)
            for k in ks:
                ps, xa, xb, wdt = pss[k]
                nc.tensor.matmul(ps[0:64, 0:wdt], statA[:, 0:C],
                                 t6[0:128, xa:xb], start=True, stop=False)
                nc.tensor.matmul(ps[64:128, 0:wdt], statA[:, C:2 * C],
                                 t6[0:128, NBHP + xa:NBHP + xb],
                                 start=True, stop=False)
            # phase B: tail matmuls
            for k in ks:
                ps, xa, xb, wdt = pss[k]
                nc.tensor.matmul(ps[0:64, 0:wdt], statB[0:21, 0:C],
                                 t6[0:21, xa + 6:xb + 6], start=False, stop=True)
                nc.tensor.matmul(ps[64:128, 0:wdt], statB[0:21, C:2 * C],
                                 t6[0:21, NBHP + xa + 6:NBHP + xb + 6],
                                 start=False, stop=True)
            # eviction: ACT relu-copies even cols into the ring (the relu
            # propagates the final max-with-0 through the pool chain), DVE
            # maxes them with the PSUM odd cols
            for k in ks:
                ps, xa, xb, wdt = pss[k]
                hw2 = wdt // 2
                ho = xa // 2
                nc.scalar.activation(ering[:, ho:ho + hw2], ps[:, 0:wdt:2],
                                     mybir.ActivationFunctionType.Relu)
                nc.vector.tensor_tensor(
                    out=tring[:, ho:ho + hw2], in0=ering[:, ho:ho + hw2],
                    in1=ps[:, 1:wdt:2], op=mybir.AluOpType.max)
                finish_rows(xb)

    nc.finalize()
    return nc


_NC_CACHE = None


def _get_nc():
    global _NC_CACHE
    if _NC_CACHE is None:
        _NC_CACHE = _build_bass()
    return _NC_CACHE


def build_in_maps(update_location, feature_map, weight, gamma, beta,
                  running_mean, running_var):
    fm = np.asarray(feature_map, np.float32)
    loc = np.asarray(update_location).astype(np.int64)
    wt = np.asarray(weight, np.float32)

    fm_pad = np.pad(fm, ((PAD, PAD), (PAD, PAD), (0, 0)))          # [646,646,3]
    # stripes B_T[t=(i,ch), r, c] = fm_pad[r+i, c, ch], r in 0..640 (row 640 pad)
    bt = np.zeros((21, H + 1, WPAD), np.float32)
    for i in range(K):
        for ch in range(CIN):
            bt[i * CIN + ch, 0:H, :] = fm_pad[i:i + H, :, ch]
    bt = bt.astype(ml_dtypes.bfloat16)

    # inactive flag = 1 where no site; indexed by output pixel (r, c) at
    # position c in the 646-pitch row; columns 640..645 stay inactive.
    flag = np.ones((H + 1, WPAD), np.float32)
    flag[loc[:, 0], loc[:, 1]] = 0.0
    flag[:, H:] = 1.0
    flag = flag.astype(ml_dtypes.bfloat16)

    # reordered weights W_re[(j,i,ch), o] = weight[i, j, ch, o]
    w_re = np.ascontiguousarray(
        wt.transpose(1, 0, 2, 3).reshape(147, COUT)).astype(np.float32)

    ones_half = np.ones((HROWS, WPAD), ml_dtypes.bfloat16)

    def build_half(r0):
        t6 = np.zeros((128, HROWS, WPAD), ml_dtypes.bfloat16)
        for j in range(6):
            sl = bt[:, r0:r0 + HROWS, :]
            t6[j * 21:(j + 1) * 21, :, :-j or None] = sl[:, :, j:]
        t6[126] = flag[r0:r0 + HROWS]
        t6[127] = ones_half
        t6p = np.zeros((128, NBHP), ml_dtypes.bfloat16)
        t6p[:, :NBH] = t6.reshape(128, NBH)
        return t6p

    # host-folded BN: inv = gamma*rsqrt(var+eps), bias = beta - mean*inv
    inv = (np.asarray(gamma, np.float32) /
           np.sqrt(np.asarray(running_var, np.float32) + BN_EPS))
    bias = np.asarray(beta, np.float32) - np.asarray(running_mean, np.float32) * inv
    sa = np.zeros((128, COUT), np.float32)
    sa[0:126] = w_re[0:126] * inv[None, :]
    sa[126] = NEG          # flag row: inactive pixels -> -1e9
    sa[127] = bias         # ones row: + BN bias
    statA = np.ascontiguousarray(
        np.concatenate([sa, sa], axis=1).astype(ml_dtypes.bfloat16))
    sb = w_re[126:147] * inv[None, :]
    statB = np.ascontiguousarray(
        np.concatenate([sb, sb], axis=1).astype(ml_dtypes.bfloat16))

    in_maps = []
    for k in range(NCORES):
        r0 = 80 * k
        t6p = np.concatenate([build_half(r0), build_half(r0 + 40)], axis=1)
        in_maps.append({"t6": np.ascontiguousarray(t6p),
                        "statA": statA, "statB": statB})
    return in_maps


def kernel(update_location, feature_map, weight, gamma, beta, running_mean,
           running_var):
    from concourse.bass_utils import run_bass_kernel_spmd

    in_maps = build_in_maps(update_location, feature_map, weight, gamma, beta,
                            running_mean, running_var)
    nc = _get_nc()
    res = run_bass_kernel_spmd(nc, in_maps, core_ids=list(range(NCORES)))
    # per-core out is [128, HPOOL*320] bf16: partition (h*64+ch), free (p*320+q)
    parts = []
    for k in range(NCORES):
        o = np.asarray(res.results[k]["out"], np.float32).reshape(
            2, COUT, HPOOL, 320)
        parts.append(o.transpose(0, 2, 3, 1).reshape(2 * HPOOL, 320, COUT)[:, :QCOLS, :])
    out = np.concatenate(parts, axis=0)[:QCOLS]
    return np.ascontiguousarray(out).astype(np.float32)
